# revision 30
# baseline (speedup 1.0000x reference)
"""Online Normalization forward (nn_Norm1d) on 8 Trainium2 NeuronCores.

Reference recurrence over the batch dim t (per feature, sequential):
    d_t   = x_t - mu^{(t)}
    y_t   = d_t / sqrt(var^{(t)} + eps)
    mu^{(t+1)}  = a*mu^{(t)}  + (1-a)*x_t
    var^{(t+1)} = a*var^{(t)} + a*(1-a)*d_t^2

Sharding: tensor-parallel over the feature dim L (4096 -> 8 x 512); each
feature's scan over N=8192 is independent, so no cross-core communication.

Per-core algorithm ("lag-bridge pairs"): time lives on SBUF partitions
(B=128 steps per block, 64 blocks processed as 32 PAIRS), features on the
free dim.  Triangular matrices on the tensor engine evaluate a whole block
of the recurrence per matmul.  The carry state (mu_base, var_base) hops
only once per PAIR: the odd block of each pair receives the even block's
contribution through constant "bridge" matmuls (M2 on x, MV2 on e) instead
of a materialized carry, which
  - halves the number of vector-engine carry row-ops (the per-instruction
    cost of a [1,512] op equals a [128,512] op - cost is free-size bound),
  - shortens the serial carry cycle to one matmul + one stt per 256 rows.

Carry extraction avoids the old WCX/TVC matmuls entirely:
  mu_base'  = x_127 - a*d_127          (one stt from psum_d row 127)
  var_base' = a*var_127 + a(1-a)d_127^2 (one stt from psum_v row 127)
To make both stts a single (x*a)+y form:
  - column 127 of WD/CD2/M2 is negated so psum_d row 127 holds -d_127;
    the y multiply uses a per-partition sign column (stt op0=mult with a
    [128,1] scalar AP) to undo the flip at no extra cost,
  - the scalar-engine square writes e' = a(1-a)*d^2 (activation scale),
    so the var stt is (var*a)+e' and the TV stationary is rescaled.

Injection matmuls (carry rows into psum) use K=32 stationaries whose only
nonzero row is row 0 - smaller LDWEIGHTS, same stream cost.
"""

import sys

for _p in ("/opt/trn_rl_repo", "/root/.axon_site/_ro/trn_rl_repo"):
    if _p not in sys.path:
        sys.path.append(_p)

import numpy as np

import concourse.bacc as bacc
import concourse.mybir as mybir
from concourse.tile import TileContext
from concourse import bass_utils

N_ROWS = 8192
L_FULL = 4096
N_CORES = 8
L_SHARD = L_FULL // N_CORES

AFWD = 0.999
EPS = 1e-05
B = 128  # time steps per block; blocks are processed in pairs (256 rows)

F32 = mybir.dt.float32
F16 = mybir.dt.float16
AF = mybir.ActivationFunctionType
ALU = mybir.AluOpType

# activation scale so Square() emits e' = a*(1-a)*d^2
SQ_SCALE = float(np.sqrt(AFWD * (1.0 - AFWD)))


def _build_weights():
    A = AFWD
    kp = A ** np.arange(B, dtype=np.float64)          # A^k
    # WD[j,k]: within-block x->d map (d_k = x_k - (1-A)*sum_j A^{k-1-j} x_j)
    WD = np.zeros((B, B), dtype=np.float64)
    for k in range(B):
        WD[k, k] += 1.0
        for j in range(k):
            WD[j, k] -= (1 - A) * A ** (k - 1 - j)
    # M2[j,k]: bridge, x(even) contribution to d of the odd block:
    #   -A^k * (1-A) * A^{127-j}
    j = np.arange(B, dtype=np.float64)
    M2 = -np.outer((1 - A) * A ** (127 - j), kp)
    # TVS[j,k]: e' -> var map with e' = A(1-A)d^2 prescaled: A^{k-1-j}, j<k
    TVS = np.zeros((B, B), dtype=np.float64)
    for k in range(B):
        for jj in range(k):
            TVS[jj, k] = A ** (k - 1 - jj)
    # MV2[j,k]: bridge, e'(even) contribution to var of the odd block
    MV2 = np.outer(A ** (127 - j), kp)
    # K=128 carry-injection stationaries (K<128 matmuls drag the PE below
    # its full-rate clock, so keep the contraction full even though only
    # one row is nonzero).  The carry lives in ztile row 31: the carry stt
    # processes the quadrant-aligned psum window [96:128] and writes ztile
    # rows 0:32, so the real carry - source row 127 - lands at row 31;
    # rows 0:30 hold junk and rows 32:127 stay zero, both ignored by the
    # zero stationary rows.
    ZR = 31
    CD = np.zeros((B, B), dtype=np.float64)
    CD[ZR, :] = -kp                   # even block: -A^k * mu_base
    CD2 = np.zeros((B, B), dtype=np.float64)
    CD2[ZR, :] = -(A ** B) * kp       # odd block: -A^{128+k} * mu_base
    CVI = np.zeros((B, B), dtype=np.float64)
    CVI[ZR, :] = kp                   # even block: +A^k * var_base
    CVI2 = np.zeros((B, B), dtype=np.float64)
    CVI2[ZR, :] = (A ** B) * kp       # odd block: +A^{128+k} * var_base
    return {"wd": WD, "m2": M2, "tvs": TVS, "mv2": MV2,
            "cd": CD, "cd2": CD2, "cvi": CVI, "cvi2": CVI2}


_WEIGHTS = {k: np.ascontiguousarray(v.astype(np.float16))
            for k, v in _build_weights().items()}


def _build_nc(n_rows: int, l_cols: int):
    assert n_rows % (2 * B) == 0
    n_pairs = n_rows // (2 * B)

    nc = bacc.Bacc()
    x = nc.declare_dram_parameter("x", [n_rows, l_cols], F32, isOutput=False)
    mu0 = nc.declare_dram_parameter("mu0", [1, l_cols], F32, isOutput=False)
    var0 = nc.declare_dram_parameter("var0", [1, l_cols], F32, isOutput=False)
    wts = {
        name: nc.declare_dram_parameter(name, list(w.shape), F16,
                                        isOutput=False)
        for name, w in _WEIGHTS.items()
    }
    y = nc.declare_dram_parameter("y", [n_rows, l_cols], F32, isOutput=True)

    with TileContext(nc) as tc:
        with (
            tc.tile_pool(name="consts", bufs=1) as cpool,
            tc.tile_pool(name="xin", bufs=4) as xpool,
            tc.tile_pool(name="esq", bufs=4) as epool,
            tc.tile_pool(name="rsy", bufs=6) as wpool,
            tc.tile_pool(name="carry", bufs=6) as zpool,
            tc.tile_pool(name="psd", bufs=3, space="PSUM") as psd,
            tc.tile_pool(name="psv", bufs=1, space="PSUM") as psv,
        ):
            wsb = {}
            for name, w in _WEIGHTS.items():
                wsb[name] = cpool.tile(list(w.shape), F16,
                                       tag=name, name=f"w_{name}")
                nc.sync.dma_start(out=wsb[name][:, :], in_=wts[name][:, :])
            eps_sb = cpool.tile([128, 1], F32, tag="eps")
            nc.vector.memset(eps_sb[:, :], EPS)

            NZ = 3
            zmu = [zpool.tile([B, l_cols], F16, tag=f"zmu{i}",
                              name=f"zmu{i}", bufs=1) for i in range(NZ)]
            zv = [zpool.tile([B, l_cols], F16, tag=f"zv{i}",
                             name=f"zv{i}", bufs=1) for i in range(NZ)]
            for i in range(NZ):
                nc.vector.memset(zmu[i][:, :], 0.0)
                nc.vector.memset(zv[i][:, :], 0.0)
            nc.gpsimd.dma_start(out=zmu[0][31:32, :], in_=mu0[:, :])
            nc.gpsimd.dma_start(out=zv[0][31:32, :], in_=var0[:, :])

            # pipeline state from the previous pair (v-chain lags one pair)
            prev = None  # (pi, psd0, psd1, e0, e1, xt)

            def emit_v_front(st):
                """v-matmuls + var carry + rs for pair st (runs one pair
                behind the d-chain)."""
                (pi, d0, d1, e0, e1, xt) = st
                zvt = zv[pi % NZ]
                # single fused v-psum (2 banks, ONE buffer): v1 in the left
                # half, v0 in the right.  Its readers (rs, var carry) all
                # finish within this iteration, so one buffer suffices and
                # frees two banks for a deeper psd rotation.
                vv = psv.tile([B, 2 * l_cols], F32, tag="psv")
                v1 = vv[:, 0:l_cols]
                v0 = vv[:, l_cols:2 * l_cols]
                # odd block first: v1 feeds the var carry for the next pair
                nc.tensor.matmul(v1, wsb["tvs"][:, :], e1[:, :],
                                 start=True, stop=False)
                nc.tensor.matmul(v1, wsb["mv2"][:, :], e0[:, :],
                                 start=False, stop=False)
                nc.tensor.matmul(v1, wsb["cvi2"][:, :], zvt[:, :],
                                 start=False, stop=True)
                nc.tensor.matmul(v0, wsb["tvs"][:, :], e0[:, :],
                                 start=True, stop=False)
                nc.tensor.matmul(v0, wsb["cvi"][:, :], zvt[:, :],
                                 start=False, stop=True)
                # var carry first on DVE: it gates the next pair's v-injects.
                # Quadrant-aligned window [96:128] -> ztile rows 0:32; the
                # real carry (psum row 127) lands at ztile row 31.
                if pi < n_pairs - 1:
                    nc.vector.scalar_tensor_tensor(
                        zv[(pi + 1) % NZ][0:32, :], vv[96:128, 0:l_cols], AFWD,
                        e1[96:128, :], ALU.mult, ALU.add)
                rs1 = wpool.tile([B, l_cols], F16, tag="rs1")
                nc.scalar.activation(rs1[:, :], v1,
                                     AF.Abs_reciprocal_sqrt, bias=eps_sb[:, :])
                rs0 = wpool.tile([B, l_cols], F16, tag="rs0")
                nc.scalar.activation(rs0[:, :], v0,
                                     AF.Abs_reciprocal_sqrt, bias=eps_sb[:, :])
                return rs0, rs1

            def emit_y(st, rs0, rs1):
                """y multiplies + stores for pair st.  Emitted after the
                next pair's mu carry so the DVE queue keeps the carry loop
                short: [var-stt, mu-stt, y1, y0]."""
                (pi, d0, d1, e0, e1, xt) = st
                y1 = wpool.tile([B, l_cols], F32, tag="y1")
                nc.vector.tensor_mul(y1[:, :], d1[:, :], rs1[:, :])
                nc.sync.dma_start(out=y[(2 * pi + 1) * B:(2 * pi + 2) * B, :],
                                  in_=y1[:, :])
                y0 = wpool.tile([B, l_cols], F32, tag="y0")
                nc.vector.tensor_mul(y0[:, :], d0[:, :], rs0[:, :])
                nc.sync.dma_start(out=y[(2 * pi) * B:(2 * pi + 1) * B, :],
                                  in_=y0[:, :])

            def issue_x_dma(pi):
                # x for both blocks of the pair; odd block in cols 512:1024
                xt = xpool.tile([B, 2 * l_cols], F16, tag="xt")
                nc.gpsimd.dma_start(out=xt[:, 0:l_cols],
                                    in_=x[2 * pi * B:(2 * pi + 1) * B, :])
                nc.gpsimd.dma_start(out=xt[:, l_cols:2 * l_cols],
                                    in_=x[(2 * pi + 1) * B:(2 * pi + 2) * B, :])
                return xt

            xt_next = issue_x_dma(0)
            for pi in range(n_pairs):
                xt = xt_next
                if pi + 1 < n_pairs:
                    xt_next = issue_x_dma(pi + 1)

                # previous pair's v-chain first: its inputs are already
                # ready, so every engine's queue starts with runnable work
                rs_prev = None
                if prev is not None:
                    rs_prev = emit_v_front(prev)

                zmt = zmu[pi % NZ]
                d0 = psd.tile([B, l_cols], F32, tag="psd0")
                d1 = psd.tile([B, l_cols], F32, tag="psd1")
                # odd block first: d1 feeds the mu carry and, via sq1, the
                # first v-matmul of the next iteration
                nc.tensor.matmul(d1[:, :], wsb["wd"][:, :],
                                 xt[:, l_cols:2 * l_cols],
                                 start=True, stop=False)
                nc.tensor.matmul(d1[:, :], wsb["m2"][:, :], xt[:, 0:l_cols],
                                 start=False, stop=False)
                nc.tensor.matmul(d1[:, :], wsb["cd2"][:, :], zmt[:, :],
                                 start=False, stop=True)
                nc.tensor.matmul(d0[:, :], wsb["wd"][:, :], xt[:, 0:l_cols],
                                 start=True, stop=False)
                nc.tensor.matmul(d0[:, :], wsb["cd"][:, :], zmt[:, :],
                                 start=False, stop=True)

                # mu carry: gates the next pair's d-injects (aligned window)
                # zmu' = x_127 - A*d_127 = (d*(-A)) + x
                if pi < n_pairs - 1:
                    nc.vector.scalar_tensor_tensor(
                        zmu[(pi + 1) % NZ][0:32, :], d1[96:128, :], -AFWD,
                        xt[96:128, l_cols:2 * l_cols], ALU.mult, ALU.add)

                # previous pair's y work comes after this pair's mu carry
                if prev is not None:
                    emit_y(prev, *rs_prev)

                e1 = epool.tile([B, l_cols], F16, tag="e1")
                nc.scalar.activation(e1[:, :], d1[:, :], AF.Square,
                                     bias=0.0, scale=SQ_SCALE)
                e0 = epool.tile([B, l_cols], F16, tag="e0")
                nc.scalar.activation(e0[:, :], d0[:, :], AF.Square,
                                     bias=0.0, scale=SQ_SCALE)

                prev = (pi, d0, d1, e0, e1, xt)

            rs_last = emit_v_front(prev)
            emit_y(prev, *rs_last)

    nc.compile()
    return nc


_NC_CACHE = {}


def _get_nc():
    key = (N_ROWS, L_SHARD)
    if key not in _NC_CACHE:
        _NC_CACHE[key] = _build_nc(*key)
    return _NC_CACHE[key]


def kernel(x, mu0, var0, _want_time=False, _trace=False):
    x = np.ascontiguousarray(np.asarray(x), dtype=np.float32)
    mu0 = np.ascontiguousarray(np.asarray(mu0), dtype=np.float32).reshape(1, -1)
    var0 = np.ascontiguousarray(np.asarray(var0), dtype=np.float32).reshape(1, -1)
    assert x.shape == (N_ROWS, L_FULL), x.shape

    nc = _get_nc()
    in_maps = []
    for c in range(N_CORES):
        sl = slice(c * L_SHARD, (c + 1) * L_SHARD)
        in_maps.append({
            "x": np.ascontiguousarray(x[:, sl]),
            "mu0": np.ascontiguousarray(mu0[:, sl]),
            "var0": np.ascontiguousarray(var0[:, sl]),
            **_WEIGHTS,
        })

    exec_ns = None
    if _trace:
        orig_upload = bass_utils.upload_artifacts
        bass_utils.upload_artifacts = lambda tmpdir: "(skipped)"
        try:
            res = bass_utils.run_bass_kernel_spmd(
                nc, in_maps, list(range(N_CORES)), trace=True
            )
            exec_ns = res.exec_time_ns
        finally:
            bass_utils.upload_artifacts = orig_upload
    else:
        res = bass_utils.run_bass_kernel_spmd(nc, in_maps, list(range(N_CORES)))

    out = np.concatenate(
        [res.results[c]["y"] for c in range(N_CORES)], axis=1
    ).astype(np.float32, copy=False)
    if _want_time:
        return out, exec_ns
    return out


# revision 31
# speedup vs baseline: 1.4319x; 1.4319x over previous
"""Online Normalization forward (nn_Norm1d) on 8 Trainium2 NeuronCores.

Reference recurrence over the batch dim t (per feature, sequential):
    d_t   = x_t - mu^{(t)}
    y_t   = d_t / sqrt(var^{(t)} + eps)
    mu^{(t+1)}  = a*mu^{(t)}  + (1-a)*x_t
    var^{(t+1)} = a*var^{(t)} + a*(1-a)*d_t^2

Sharding: tensor-parallel over the feature dim L (4096 -> 8 x 512); each
feature's scan over N=8192 is independent, so no cross-core communication.

Per-core algorithm ("lag-bridge pairs"): time lives on SBUF partitions
(B=128 steps per block, 64 blocks processed as 32 PAIRS), features on the
free dim.  Triangular matrices on the tensor engine evaluate a whole block
of the recurrence per matmul.  The carry state (mu_base, var_base) hops
only once per PAIR: the odd block of each pair receives the even block's
contribution through constant "bridge" matmuls (M2 on x, MV2 on e) instead
of a materialized carry, which
  - halves the number of vector-engine carry row-ops (the per-instruction
    cost of a [1,512] op equals a [128,512] op - cost is free-size bound),
  - shortens the serial carry cycle to one matmul + one stt per 256 rows.

Carry extraction avoids the old WCX/TVC matmuls entirely:
  mu_base'  = x_127 - a*d_127          (one stt from psum_d row 127)
  var_base' = a*var_127 + a(1-a)d_127^2 (one stt from psum_v row 127)
To make both stts a single (x*a)+y form:
  - column 127 of WD/CD2/M2 is negated so psum_d row 127 holds -d_127;
    the y multiply uses a per-partition sign column (stt op0=mult with a
    [128,1] scalar AP) to undo the flip at no extra cost,
  - the scalar-engine square writes e' = a(1-a)*d^2 (activation scale),
    so the var stt is (var*a)+e' and the TV stationary is rescaled.

Injection matmuls (carry rows into psum) use K=32 stationaries whose only
nonzero row is row 0 - smaller LDWEIGHTS, same stream cost.
"""

import sys

for _p in ("/opt/trn_rl_repo", "/root/.axon_site/_ro/trn_rl_repo"):
    if _p not in sys.path:
        sys.path.append(_p)

import numpy as np

import concourse.bacc as bacc
import concourse.mybir as mybir
from concourse.tile import TileContext
from concourse import bass_utils

N_ROWS = 8192
L_FULL = 4096
N_CORES = 8
L_SHARD = L_FULL // N_CORES

AFWD = 0.999
EPS = 1e-05
B = 128  # time steps per block; blocks are processed in pairs (256 rows)

F32 = mybir.dt.float32
F16 = mybir.dt.float16
AF = mybir.ActivationFunctionType
ALU = mybir.AluOpType

# activation scale so Square() emits e' = a*(1-a)*d^2
SQ_SCALE = float(np.sqrt(AFWD * (1.0 - AFWD)))


def _build_weights():
    A = AFWD
    kp = A ** np.arange(B, dtype=np.float64)          # A^k
    # WD[j,k]: within-block x->d map (d_k = x_k - (1-A)*sum_j A^{k-1-j} x_j)
    WD = np.zeros((B, B), dtype=np.float64)
    for k in range(B):
        WD[k, k] += 1.0
        for j in range(k):
            WD[j, k] -= (1 - A) * A ** (k - 1 - j)
    # M2[j,k]: bridge, x(even) contribution to d of the odd block:
    #   -A^k * (1-A) * A^{127-j}
    j = np.arange(B, dtype=np.float64)
    M2 = -np.outer((1 - A) * A ** (127 - j), kp)
    # TVS[j,k]: e' -> var map with e' = A(1-A)d^2 prescaled: A^{k-1-j}, j<k
    TVS = np.zeros((B, B), dtype=np.float64)
    for k in range(B):
        for jj in range(k):
            TVS[jj, k] = A ** (k - 1 - jj)
    # MV2[j,k]: bridge, e'(even) contribution to var of the odd block
    MV2 = np.outer(A ** (127 - j), kp)
    # K=128 carry-injection stationaries (K<128 matmuls drag the PE below
    # its full-rate clock, so keep the contraction full even though only
    # one row is nonzero).  The carry lives in ztile row 31: the carry stt
    # processes the quadrant-aligned psum window [96:128] and writes ztile
    # rows 0:32, so the real carry - source row 127 - lands at row 31;
    # rows 0:30 hold junk and rows 32:127 stay zero, both ignored by the
    # zero stationary rows.
    ZR = 31
    CD = np.zeros((B, B), dtype=np.float64)
    CD[ZR, :] = -kp                   # even block: -A^k * mu_base
    CD2 = np.zeros((B, B), dtype=np.float64)
    CD2[ZR, :] = -(A ** B) * kp       # odd block: -A^{128+k} * mu_base
    CVI = np.zeros((B, B), dtype=np.float64)
    CVI[ZR, :] = kp                   # even block: +A^k * var_base
    CVI2 = np.zeros((B, B), dtype=np.float64)
    CVI2[ZR, :] = (A ** B) * kp       # odd block: +A^{128+k} * var_base
    return {"wd": WD, "m2": M2, "tvs": TVS, "mv2": MV2,
            "cd": CD, "cd2": CD2, "cvi": CVI, "cvi2": CVI2}


_WEIGHTS = {k: np.ascontiguousarray(v.astype(np.float16))
            for k, v in _build_weights().items()}


def _build_nc(n_rows: int, l_cols: int):
    assert n_rows % (2 * B) == 0
    n_pairs = n_rows // (2 * B)

    nc = bacc.Bacc()
    x = nc.declare_dram_parameter("x", [n_rows, l_cols], F32, isOutput=False)
    mu0 = nc.declare_dram_parameter("mu0", [1, l_cols], F32, isOutput=False)
    var0 = nc.declare_dram_parameter("var0", [1, l_cols], F32, isOutput=False)
    wts = {
        name: nc.declare_dram_parameter(name, list(w.shape), F16,
                                        isOutput=False)
        for name, w in _WEIGHTS.items()
    }
    y = nc.declare_dram_parameter("y", [n_rows, l_cols], F32, isOutput=True)

    with TileContext(nc) as tc:
        with (
            tc.tile_pool(name="consts", bufs=1) as cpool,
            tc.tile_pool(name="xin", bufs=4) as xpool,
            tc.tile_pool(name="esq", bufs=4) as epool,
            tc.tile_pool(name="rsy", bufs=6) as wpool,
            tc.tile_pool(name="carry", bufs=6) as zpool,
            tc.tile_pool(name="psd", bufs=3, space="PSUM") as psd,
            tc.tile_pool(name="psv", bufs=1, space="PSUM") as psv,
        ):
            wsb = {}
            for name, w in _WEIGHTS.items():
                wsb[name] = cpool.tile(list(w.shape), F16,
                                       tag=name, name=f"w_{name}")
                nc.sync.dma_start(out=wsb[name][:, :], in_=wts[name][:, :])
            eps_sb = cpool.tile([128, 1], F32, tag="eps")
            nc.vector.memset(eps_sb[:, :], EPS)

            NZ = 3
            zmu = [zpool.tile([B, l_cols], F16, tag=f"zmu{i}",
                              name=f"zmu{i}", bufs=1) for i in range(NZ)]
            zv = [zpool.tile([B, l_cols], F16, tag=f"zv{i}",
                             name=f"zv{i}", bufs=1) for i in range(NZ)]
            for i in range(NZ):
                nc.vector.memset(zmu[i][:, :], 0.0)
                nc.vector.memset(zv[i][:, :], 0.0)
            nc.gpsimd.dma_start(out=zmu[0][31:32, :], in_=mu0[:, :])
            nc.gpsimd.dma_start(out=zv[0][31:32, :], in_=var0[:, :])

            # pipeline state from the previous pair (v-chain lags one pair)
            prev = None  # (pi, psd0, psd1, e0, e1, xt)

            def emit_v_front(st):
                """v-matmuls + var carry + rs for pair st (runs one pair
                behind the d-chain)."""
                (pi, d0, d1, e0, e1, xt) = st
                zvt = zv[pi % NZ]
                # psv tiles are single-buffered per tag: all their readers
                # (rs, var carry) complete within this iteration, and the
                # freed banks give psd a 3-deep rotation instead.
                v1 = psv.tile([B, l_cols], F32, tag="psv1")
                v0 = psv.tile([B, l_cols], F32, tag="psv0")
                # odd block first: v1 feeds the var carry for the next pair
                nc.tensor.matmul(v1[:, :], wsb["tvs"][:, :], e1[:, :],
                                 start=True, stop=False)
                nc.tensor.matmul(v1[:, :], wsb["mv2"][:, :], e0[:, :],
                                 start=False, stop=False)
                nc.tensor.matmul(v1[:, :], wsb["cvi2"][:, :], zvt[:, :],
                                 start=False, stop=True)
                nc.tensor.matmul(v0[:, :], wsb["tvs"][:, :], e0[:, :],
                                 start=True, stop=False)
                nc.tensor.matmul(v0[:, :], wsb["cvi"][:, :], zvt[:, :],
                                 start=False, stop=True)
                # var carry first on DVE: it gates the next pair's v-injects.
                # Quadrant-aligned window [96:128] -> ztile rows 0:32; the
                # real carry (psum row 127) lands at ztile row 31.
                if pi < n_pairs - 1:
                    nc.vector.scalar_tensor_tensor(
                        zv[(pi + 1) % NZ][0:32, :], v1[96:128, :], AFWD,
                        e1[96:128, :], ALU.mult, ALU.add)
                rs1 = wpool.tile([B, l_cols], F16, tag="rs1")
                nc.scalar.activation(rs1[:, :], v1[:, :],
                                     AF.Abs_reciprocal_sqrt, bias=eps_sb[:, :])
                rs0 = wpool.tile([B, l_cols], F16, tag="rs0")
                nc.scalar.activation(rs0[:, :], v0[:, :],
                                     AF.Abs_reciprocal_sqrt, bias=eps_sb[:, :])
                return rs0, rs1

            def emit_y(st, rs0, rs1):
                """y multiplies + stores for pair st.  Emitted after the
                next pair's mu carry so the DVE queue keeps the carry loop
                short: [var-stt, mu-stt, y1, y0]."""
                (pi, d0, d1, e0, e1, xt) = st
                y1 = wpool.tile([B, l_cols], F32, tag="y1")
                nc.vector.tensor_mul(y1[:, :], d1[:, :], rs1[:, :])
                nc.sync.dma_start(out=y[(2 * pi + 1) * B:(2 * pi + 2) * B, :],
                                  in_=y1[:, :])
                y0 = wpool.tile([B, l_cols], F32, tag="y0")
                nc.vector.tensor_mul(y0[:, :], d0[:, :], rs0[:, :])
                nc.sync.dma_start(out=y[(2 * pi) * B:(2 * pi + 1) * B, :],
                                  in_=y0[:, :])

            def issue_x_dma(pi):
                # x for both blocks of the pair; odd block in cols 512:1024
                xt = xpool.tile([B, 2 * l_cols], F16, tag="xt")
                nc.gpsimd.dma_start(out=xt[:, 0:l_cols],
                                    in_=x[2 * pi * B:(2 * pi + 1) * B, :])
                nc.gpsimd.dma_start(out=xt[:, l_cols:2 * l_cols],
                                    in_=x[(2 * pi + 1) * B:(2 * pi + 2) * B, :])
                return xt

            xt_next = issue_x_dma(0)
            for pi in range(n_pairs):
                xt = xt_next
                if pi + 1 < n_pairs:
                    xt_next = issue_x_dma(pi + 1)

                # previous pair's v-chain first: its inputs are already
                # ready, so every engine's queue starts with runnable work
                rs_prev = None
                if prev is not None:
                    rs_prev = emit_v_front(prev)

                zmt = zmu[pi % NZ]
                d0 = psd.tile([B, l_cols], F32, tag="psd0")
                d1 = psd.tile([B, l_cols], F32, tag="psd1")
                # odd block first: d1 feeds the mu carry and, via sq1, the
                # first v-matmul of the next iteration
                nc.tensor.matmul(d1[:, :], wsb["wd"][:, :],
                                 xt[:, l_cols:2 * l_cols],
                                 start=True, stop=False)
                nc.tensor.matmul(d1[:, :], wsb["m2"][:, :], xt[:, 0:l_cols],
                                 start=False, stop=False)
                nc.tensor.matmul(d1[:, :], wsb["cd2"][:, :], zmt[:, :],
                                 start=False, stop=True)
                nc.tensor.matmul(d0[:, :], wsb["wd"][:, :], xt[:, 0:l_cols],
                                 start=True, stop=False)
                nc.tensor.matmul(d0[:, :], wsb["cd"][:, :], zmt[:, :],
                                 start=False, stop=True)

                # mu carry: gates the next pair's d-injects (aligned window)
                # zmu' = x_127 - A*d_127 = (d*(-A)) + x
                if pi < n_pairs - 1:
                    nc.vector.scalar_tensor_tensor(
                        zmu[(pi + 1) % NZ][0:32, :], d1[96:128, :], -AFWD,
                        xt[96:128, l_cols:2 * l_cols], ALU.mult, ALU.add)

                # previous pair's y work comes after this pair's mu carry
                if prev is not None:
                    emit_y(prev, *rs_prev)

                e1 = epool.tile([B, l_cols], F16, tag="e1")
                nc.scalar.activation(e1[:, :], d1[:, :], AF.Square,
                                     bias=0.0, scale=SQ_SCALE)
                e0 = epool.tile([B, l_cols], F16, tag="e0")
                nc.scalar.activation(e0[:, :], d0[:, :], AF.Square,
                                     bias=0.0, scale=SQ_SCALE)

                prev = (pi, d0, d1, e0, e1, xt)

            rs_last = emit_v_front(prev)
            emit_y(prev, *rs_last)

    nc.compile()
    return nc


_NC_CACHE = {}


def _get_nc():
    key = (N_ROWS, L_SHARD)
    if key not in _NC_CACHE:
        _NC_CACHE[key] = _build_nc(*key)
    return _NC_CACHE[key]


def kernel(x, mu0, var0, _want_time=False, _trace=False):
    x = np.ascontiguousarray(np.asarray(x), dtype=np.float32)
    mu0 = np.ascontiguousarray(np.asarray(mu0), dtype=np.float32).reshape(1, -1)
    var0 = np.ascontiguousarray(np.asarray(var0), dtype=np.float32).reshape(1, -1)
    assert x.shape == (N_ROWS, L_FULL), x.shape

    nc = _get_nc()
    in_maps = []
    for c in range(N_CORES):
        sl = slice(c * L_SHARD, (c + 1) * L_SHARD)
        in_maps.append({
            "x": np.ascontiguousarray(x[:, sl]),
            "mu0": np.ascontiguousarray(mu0[:, sl]),
            "var0": np.ascontiguousarray(var0[:, sl]),
            **_WEIGHTS,
        })

    exec_ns = None
    if _trace:
        orig_upload = bass_utils.upload_artifacts
        bass_utils.upload_artifacts = lambda tmpdir: "(skipped)"
        try:
            res = bass_utils.run_bass_kernel_spmd(
                nc, in_maps, list(range(N_CORES)), trace=True
            )
            exec_ns = res.exec_time_ns
        finally:
            bass_utils.upload_artifacts = orig_upload
    else:
        res = bass_utils.run_bass_kernel_spmd(nc, in_maps, list(range(N_CORES)))

    out = np.concatenate(
        [res.results[c]["y"] for c in range(N_CORES)], axis=1
    ).astype(np.float32, copy=False)
    if _want_time:
        return out, exec_ns
    return out
